# revision 1
# baseline (speedup 1.0000x reference)
"""Trainium2 Bass kernel for nn_BLP_52467320487972 (retrieval_knn, L1 scores).

score[b, e] = -sum_d |query_sum[b, d] - E_embed[e, d]|,  E_embed = [other_emb[0]; ent_pkl @ proj_W.T]

Strategy (8 NeuronCores, entity-sharded, 5000(+pad) entities/core):
  host:   exact query_sum [32, 256] (tiny gather + normalize); score column 0;
          per-core ent_pkl shard transposed to [768, 5120] bf16.
  device: uses the identity sum|x| = 2*sum(relu(x)) - sum(x) so the vector
          engine runs at its 4x tensor_scalar mode and the PE does all
          reductions.  Lanes over the 32 query rows:
    * "transposed" lanes work on P.T tiles [128d, 1024e] that the PE produces
      directly (lhsT=W chunk, rhs=A chunk; no on-chip transposes):
        - DVE lane: relu(P - q[b]) via one fused tensor_scalar
          (op0=subtract scalar-per-partition, op1=max 0) at the 4x perf mode;
        - ACT lane: activation(Relu, bias=-q[:,b]) (per-partition bias works
          in this d-major layout);
      a PE matmul with a one-column-of-2.0 "staircase" lhsT column-sums each
      relu tile into PSUM row b; a shared all-(-1) lhsT matmul subtracts
      colsum(P.T) from every row (the "- sum x" term; q's rowsum added on
      host).
    * custom lane: entity-major P tiles [128e, 256d]; one custom fused DVE op
      per (tile, b) computing |q - p| with free-dim accumulate (keeps a few
      rows off the PE-reduction critical path).
  host:   stitch the two device outputs, negate, prepend column 0.
"""

import sys

for _p in ("/opt/trn_rl_repo", "/root/.axon_site/_ro/trn_rl_repo"):
    if _p not in sys.path:
        sys.path.append(_p)

import numpy as np
import ml_dtypes

NUM_ENT = 40000
NUM_REL = 100
EMBED_DIM = 256
FEAT_DIM = 768
BATCH = 32
N_CORES = 8
SHARD = NUM_ENT // N_CORES          # 5000
SHARD_PAD = 5120                    # 40 tiles of 128
N_ETILES = SHARD_PAD // 128         # 40
K_CHUNKS = FEAT_DIM // 128          # 6
GROUP = 1024                        # entities per transposed-lane group
N_GROUPS = SHARD_PAD // GROUP       # 5
TILES_PER_GROUP = GROUP // 128      # 8
EPS = 1e-12

# Query-row assignment to lanes.
N_CUST = 0                          # custom-DVE lane (e-major, no PE reduce)
N_DVEF = 15                         # DVE rows whose relu halves get folded (TT-add)
N_DVET = 25                         # DVE tensor_scalar lane (d-major)
N_ACTT = BATCH - N_CUST - N_DVET    # ACT lane (d-major)
DVET = list(range(N_CUST, N_CUST + N_DVET))
ACTT = list(range(N_CUST + N_DVET, BATCH))

BF16 = ml_dtypes.bfloat16

_CACHE = {}


def _get_sad_op():
    """Custom DVE op: out = |in0 - in1|, accum_out = s0 + sum(out)."""
    if "sad_op" in _CACHE:
        return _CACHE["sad_op"]
    from operator import add

    from concourse import dve_ops
    from concourse.dve_spec import C0, Spec, maxx, Src0, Src1

    def _ref_sad(in0, in1, s0, s1, imm2):
        b = np.abs(in0.astype(np.float32) - in1.astype(np.float32)).astype(np.float32)
        return b, np.asarray(s0, np.float32).reshape(-1, 1) + b.reshape(
            b.shape[0], -1
        ).sum(-1, keepdims=True)

    op = dve_ops.DveOp(
        "SAD_ACC_ANT",
        Spec(
            body=maxx(Src0 - Src1, Src1 - Src0),
            accum=add,
            accum_init=C0,
            reference=_ref_sad,
        ),
        subdim=False,
        uops_sha={},
    )
    if op.name not in dve_ops._SUB_OPCODE_FOR_NAME:
        dve_ops.OPS.append(op)
        dve_ops.CUSTOM_DVE_SPECS[op.name] = op.spec
        dve_ops._SUB_OPCODE_FOR_NAME[op.name] = (
            max(dve_ops._SUB_OPCODE_FOR_NAME.values()) + 1
        )
    from concourse.dve_spec import lower
    from concourse.dve_uop import DveOpSpec

    for ver in ("v3", "v4"):
        spec = DveOpSpec(
            name=op.name,
            opcode=dve_ops.get_dve_sub_opcode(op.name),
            uops=lower(op.spec, ver=ver),
            rd1_en=True,
        )
        op.uops_sha[ver] = spec.sha(ver)
    _CACHE["sad_op"] = op
    return op


def _build_program():
    import concourse.bacc as bacc
    import concourse.mybir as mybir
    import concourse.tile as tile

    f32 = mybir.dt.float32
    bf16 = mybir.dt.bfloat16
    AL = mybir.AluOpType

    sad = _get_sad_op()
    nc = bacc.Bacc("TRN2", target_bir_lowering=False, debug=False, num_devices=N_CORES)
    a_t = nc.declare_dram_parameter("a_t", [FEAT_DIM, SHARD_PAD], bf16, isOutput=False)
    w_t = nc.declare_dram_parameter("w_t", [FEAT_DIM, EMBED_DIM], bf16, isOutput=False)
    if N_CUST:
        qb = nc.declare_dram_parameter("qb", [128, N_CUST, EMBED_DIM], bf16, isOutput=False)
    qt = nc.declare_dram_parameter("qt", [128, 2, BATCH], f32, isOutput=False)
    qtn = nc.declare_dram_parameter("qtn", [128, 2, BATCH], f32, isOutput=False)
    amask = nc.declare_dram_parameter("amask", [128, 1], f32, isOutput=False)
    st_out = nc.declare_dram_parameter("st_out", [BATCH, SHARD_PAD], f32, isOutput=True)
    if N_CUST:
        s_out = nc.declare_dram_parameter("s_out", [SHARD_PAD, N_CUST], f32, isOutput=True)

    E_CHUNK = 1280
    N_ECHUNKS = SHARD_PAD // E_CHUNK  # 4
    with tile.TileContext(nc) as tc:
        with (
            tc.tile_pool(name="const", bufs=1) as const_pool,
            tc.tile_pool(name="pt", bufs=5) as pt_pool,
            tc.tile_pool(name="p", bufs=3) as p_pool,
            tc.tile_pool(name="s", bufs=8) as s_pool,
            tc.tile_pool(name="psum", bufs=2, space="PSUM") as psum_pool,
            tc.tile_pool(name="absd", bufs=8) as absd_pool,
            tc.tile_pool(name="sr", bufs=2) as sr_pool,
            tc.tile_pool(name="psumt", bufs=2, space="PSUM") as psumt_pool,
            tc.tile_pool(name="psums", bufs=2, space="PSUM") as psums_pool,
        ):
            # ---- resident constants ----
            w_sb = const_pool.tile([128, K_CHUNKS, EMBED_DIM], bf16)
            for k in range(K_CHUNKS):
                nc.sync.dma_start(
                    out=w_sb[:, k, :], in_=w_t[128 * k : 128 * (k + 1), :]
                )
            if N_CUST:
                qb_sb = const_pool.tile([128, N_CUST, EMBED_DIM], bf16)
                nc.sync.dma_start(out=qb_sb[:], in_=qb[:])
            qt_sb = const_pool.tile([128, 2, BATCH], f32)
            nc.sync.dma_start(out=qt_sb[:], in_=qt[:])
            qtn_sb = const_pool.tile([128, 2, BATCH], f32)
            nc.sync.dma_start(out=qtn_sb[:], in_=qtn[:])
            amask_sb = const_pool.tile([128, 1], f32)
            nc.sync.dma_start(out=amask_sb[:], in_=amask[:])
            # staircase: column 31 is all-2.0; stair[:, 31-b : 63-b] is a
            # [128, 32] one-column lhsT that routes 2*column-sum into PSUM
            # row b (the 2x of score = 2*sum relu(x) - sum x).
            stair = const_pool.tile([128, 2 * BATCH - 1], bf16)
            nc.gpsimd.memset(stair[:], 0.0)
            nc.gpsimd.memset(stair[:, BATCH - 1 : BATCH], 2.0)
            negones = const_pool.tile([128, BATCH], bf16)
            nc.gpsimd.memset(negones[:], -1.0)

            a_all = const_pool.tile([128, K_CHUNKS, SHARD_PAD], bf16)
            for c in range(N_ECHUNKS):
                for k in range(K_CHUNKS):
                    nc.sync.dma_start(
                        out=a_all[:, k, E_CHUNK * c : E_CHUNK * (c + 1)],
                        in_=a_t[
                            128 * k : 128 * (k + 1), E_CHUNK * c : E_CHUNK * (c + 1)
                        ],
                    )

            # last group shrinks to the real entity count (5000 = 4*1024+904)
            gsizes = [GROUP] * (N_GROUPS - 1) + [SHARD - GROUP * (N_GROUPS - 1)]
            for g in range(N_GROUPS):
                g0 = g * GROUP
                gsz = gsizes[g]
                csz = [512] * (gsz // 512) + ([gsz % 512] if gsz % 512 else [])
                coff = [sum(csz[:i]) for i in range(len(csz))]
                # ---- transposed projection: P.T halves [128d, GROUP] ----
                pt_sb = []
                for h in range(2):
                    ptp = psumt_pool.tile([128, gsz], f32, tag="ptp")
                    for c in range(len(csz)):
                        for k in range(K_CHUNKS):
                            nc.tensor.matmul(
                                ptp[:, coff[c] : coff[c] + csz[c]],
                                w_sb[:, k, 128 * h : 128 * (h + 1)],
                                a_all[:, k, g0 + coff[c] : g0 + coff[c] + csz[c]],
                                start=(k == 0),
                                stop=(k == K_CHUNKS - 1),
                            )
                    pth = pt_pool.tile([128, gsz], bf16, tag="pt")
                    nc.scalar.copy(pth[:], ptp[:])
                    pt_sb.append(pth)

                # ---- DVE + ACT lanes -> PE column-sum reductions ----
                psum_s = psums_pool.tile([BATCH, gsz], f32, tag="psum_s")
                u16 = mybir.dt.uint16
                n_red = 2 * (N_DVET + N_ACTT)
                i_red = 0
                for b in DVET + ACTT:
                    in_dvet = b < N_CUST + N_DVET
                    fold = in_dvet and (b - N_CUST) < N_DVEF
                    halves = []
                    for h in range(2):
                        if in_dvet:
                            absd = absd_pool.tile([128, gsz], bf16, tag="absd")
                            nc.vector.tensor_scalar(
                                out=absd[:],
                                in0=pt_sb[h][:],
                                scalar1=qt_sb[:, h, b : b + 1],
                                scalar2=0.0,
                                op0=AL.subtract,
                                op1=AL.max,
                            )
                        else:
                            absd = absd_pool.tile([128, gsz], bf16, tag="absd2")
                            nc.scalar.activation(
                                absd[:],
                                pt_sb[h][:],
                                mybir.ActivationFunctionType.Relu,
                                bias=qtn_sb[:, h, b : b + 1],
                                scale=1.0,
                            )
                        halves.append(absd)
                    if fold:
                        rs = absd_pool.tile([128, gsz], bf16, tag="rsum")
                        nc.vector.tensor_add(rs[:], halves[0][:], halves[1][:])
                        reds = [rs]
                    else:
                        reds = halves
                    for r in reds:
                        for c in range(len(csz)):
                            nc.tensor.matmul(
                                psum_s[:, coff[c] : coff[c] + csz[c]],
                                stair[:, BATCH - 1 - b : 2 * BATCH - 1 - b],
                                r[:, coff[c] : coff[c] + csz[c]],
                                start=(i_red == 0),
                                stop=False,
                                skip_group_check=True,
                            )
                        i_red += 1
                for h in range(2):
                    for c in range(len(csz)):
                        nc.tensor.matmul(
                            psum_s[:, coff[c] : coff[c] + csz[c]],
                            negones[:],
                            pt_sb[h][:, coff[c] : coff[c] + csz[c]],
                            start=False,
                            stop=(h == 1),
                            skip_group_check=True,
                        )
                # ---- custom lane: entity-major tiles ----
                for t in range(TILES_PER_GROUP if N_CUST else 0):
                    e0 = g0 + 128 * t
                    psum = psum_pool.tile([128, EMBED_DIM], f32)
                    for k in range(K_CHUNKS):
                        nc.tensor.matmul(
                            psum[:],
                            a_all[:, k, e0 : e0 + 128],
                            w_sb[:, k, :],
                            start=(k == 0),
                            stop=(k == K_CHUNKS - 1),
                        )
                    p_sb = p_pool.tile([128, EMBED_DIM], bf16)
                    nc.scalar.copy(p_sb[:], psum[:])
                    s_tile = s_pool.tile([128, N_CUST], f32)
                    for b in range(N_CUST):
                        dumv = absd_pool.tile([128, EMBED_DIM], bf16, tag="dvedump")
                        nc.vector._custom_dve(
                            sad,
                            out=dumv[:],
                            in0=qb_sb[:, b, :],
                            in1=p_sb[:],
                            s0=0.0,
                            accum_out=s_tile[:, b : b + 1],
                        )
                    nc.sync.dma_start(out=s_out[e0 : e0 + 128, :], in_=s_tile[:])

                sr = sr_pool.tile([BATCH, gsz], f32, tag="sr")
                nc.scalar.copy(sr[:], psum_s[:])
                nc.sync.dma_start(out=st_out[:, g0 : g0 + gsz], in_=sr[:])

    nc.compile()
    return nc


def _get_program():
    if "nc" not in _CACHE:
        _CACHE["nc"] = _build_program()
    return _CACHE["nc"]


def _host_query_sum(ent_pkl, other_emb, proj_W, batch_input_ids, mp):
    """Exact replica of the reference's query path, on host (64 rows only)."""
    ids = np.concatenate([batch_input_ids[:, :mp], batch_input_ids[:, mp + 1 : 3]], axis=1)
    ids = ids.astype(np.int64)  # [B, 2]
    q = np.empty((BATCH, 2, EMBED_DIM), dtype=np.float32)
    for b in range(BATCH):
        for j in range(2):
            idx = int(ids[b, j])
            if idx == 0:
                row = other_emb[0]
            elif idx <= NUM_ENT:
                row = ent_pkl[idx - 1].astype(np.float32) @ proj_W.T.astype(np.float32)
            else:
                row = other_emb[idx - NUM_ENT]
            q[b, j] = row
    norm = np.sqrt((q * q).sum(-1, keepdims=True))
    q = q / np.maximum(norm, EPS)
    return q.sum(axis=1)  # [B, 256] float32


def kernel(ent_pkl, other_emb, proj_W, batch_input_ids, batch_mask_position, _timing=None):
    from concourse.bass_utils import run_bass_kernel_spmd

    ent_pkl = np.asarray(ent_pkl, dtype=np.float32)
    other_emb = np.asarray(other_emb, dtype=np.float32)
    proj_W = np.asarray(proj_W, dtype=np.float32)
    batch_input_ids = np.asarray(batch_input_ids)
    mp = int(np.asarray(batch_mask_position))

    q_sum = _host_query_sum(ent_pkl, other_emb, proj_W, batch_input_ids, mp)

    # score column 0: entity row = other_emb[0]
    col0 = -np.abs(q_sum - other_emb[0][None, :]).sum(-1)  # [B]

    # Per-core device inputs.
    w_t_np = np.ascontiguousarray(proj_W.T).astype(BF16)  # [768, 256]
    if N_CUST:
        qb_np = np.ascontiguousarray(
            np.broadcast_to(
                q_sum[:N_CUST].astype(BF16)[None, :, :], (128, N_CUST, EMBED_DIM)
            )
        )
    # qt[d, h, b] = q_sum[b, 128h + d];  qtn = -q.T in f32
    qth = np.transpose(q_sum.T.reshape(2, 128, BATCH), (1, 0, 2))  # [128, 2, 32]
    qt_np = np.ascontiguousarray(qth.astype(np.float32))
    qtn_np = np.ascontiguousarray((-qth).astype(np.float32))
    amask_np = np.full((128, 1), 0x7FFFFFFF, dtype=np.uint32).view(np.float32)
    in_maps = []
    for c in range(N_CORES):
        shard = ent_pkl[c * SHARD : (c + 1) * SHARD]  # [5000, 768]
        a_t_np = np.zeros((FEAT_DIM, SHARD_PAD), dtype=BF16)
        a_t_np[:, :SHARD] = shard.T.astype(BF16)
        m = {
            "a_t": a_t_np,
            "w_t": w_t_np,
            "qt": qt_np,
            "qtn": qtn_np,
            "amask": amask_np,
        }
        if N_CUST:
            m["qb"] = qb_np
        in_maps.append(m)

    nc = _get_program()
    kwargs = dict(_timing) if _timing else {}
    res = run_bass_kernel_spmd(nc, in_maps, list(range(N_CORES)), **kwargs)
    if _timing is not None:
        _CACHE["last_results"] = res

    qsum = q_sum.sum(-1).astype(np.float32)  # [B]
    s_ent = np.empty((BATCH, NUM_ENT), dtype=np.float32)
    for c in range(N_CORES):
        sl = slice(c * SHARD, (c + 1) * SHARD)
        s_ent[:, sl] = res.results[c]["st_out"][:, :SHARD]
        if N_CUST:
            s_ent[:N_CUST, sl] = res.results[c]["s_out"][:SHARD, :].T
    s_ent[N_CUST:] += qsum[N_CUST:, None]
    out = np.empty((BATCH, NUM_ENT + 1), dtype=np.float32)
    out[:, 0] = col0
    out[:, 1:] = -s_ent
    return out



# revision 6
# speedup vs baseline: 1.3443x; 1.3443x over previous
"""Trainium2 Bass kernel for nn_BLP_52467320487972 (retrieval_knn, L1 scores).

score[b, e] = -sum_d |query_sum[b, d] - E_embed[e, d]|,
E_embed = [other_emb[0]; ent_pkl @ proj_W.T]

Strategy (8 NeuronCores, entity-sharded, 5000(+pad) entities/core):
  host:   exact query_sum [32, 256] (tiny gather + normalize); score column 0;
          per-core ent shard pre-transposed/packed for fp8 DoubleRow matmuls.
  device: projection P.T = W.T.T @ A.T runs as fp8e4 DoubleRow matmuls
          (both operands fp8, 2 k-tiles contracted per pass at 0.5 cyc/col);
          W is pre-scaled by 32 to stay clear of fp8 subnormals and the
          PSUM->SBUF bf16 copy un-scales by 1/32.
          The 32 query rows are split across engines to balance load:
            - DVE bf16 lane: relu(P - q) via 4x tensor_scalar, PE column-sums
              each bf16 half-tile with a 2.0-stair (|x| = 2 relu(x) - x);
            - DVE fp8 lane: same relu but emitted fp8 (2x mode), reduced by
              one fp8 DoubleRow pass per row (both halves in the pair dim);
            - ACT lane: |P - q| directly via activation(Abs, bias=-q) in fp8;
            - Pool (GPSIMD) lane: relu tensor_scalar in fp8, DoubleRow reduce.
          A single negones matmul per half adds the "- sum x" correction to
          relu-lane rows only; host adds sum(q) for those rows.
  host:   stitch score columns, negate, prepend column 0.
"""

import sys

for _p in ("/opt/trn_rl_repo", "/root/.axon_site/_ro/trn_rl_repo"):
    if _p not in sys.path:
        sys.path.append(_p)

import numpy as np
import ml_dtypes

NUM_ENT = 40000
NUM_REL = 100
EMBED_DIM = 256
FEAT_DIM = 768
BATCH = 32
N_CORES = 8
SHARD = NUM_ENT // N_CORES          # 5000
SHARD_PAD = 5120                    # 40 tiles of 128
K_PAIRS = 3                         # 768 = 3 pairs * 2 * 128
GROUP = 1024
N_GROUPS = SHARD_PAD // GROUP       # 5
CHUNK = 512                         # matmul moving-dim chunk
EPS = 1e-12

# Query-row assignment (rows are contiguous so the relu/abs split is a
# simple column range in the negones matrix):
N_DVEB = 13                         # DVE bf16 relu rows (bf16 stair reduce)
N_DVE8 = 7                          # DVE fp8 relu rows (DoubleRow reduce)
N_POOL = 5                          # Pool fp8 relu rows (DoubleRow reduce)
N_ACT = BATCH - N_DVEB - N_DVE8 - N_POOL  # 7 ACT abs rows (DoubleRow reduce)
N_RELU = N_DVEB + N_DVE8 + N_POOL   # rows needing the 2relu-x identity

ROWS_DVEB = list(range(0, N_DVEB))
ROWS_DVE8 = list(range(N_DVEB, N_DVEB + N_DVE8))
ROWS_POOL = list(range(N_DVEB + N_DVE8, N_RELU))
ROWS_ACT = list(range(N_RELU, BATCH))

BF16 = ml_dtypes.bfloat16
FP8 = ml_dtypes.float8_e4m3
W_SCALE = 32.0

_CACHE = {}


def _build_program():
    import concourse.bacc as bacc
    import concourse.mybir as mybir
    import concourse.tile as tile

    f32 = mybir.dt.float32
    bf16 = mybir.dt.bfloat16
    fp8 = mybir.dt.float8e4
    AL = mybir.AluOpType
    ACT = mybir.ActivationFunctionType
    DR = mybir.MatmulPerfMode.DoubleRow

    nc = bacc.Bacc("TRN2", target_bir_lowering=False, debug=False, num_devices=N_CORES)
    a_dr = nc.declare_dram_parameter(
        "a_dr", [128, K_PAIRS, 2, SHARD_PAD], fp8, isOutput=False)
    w_dr = nc.declare_dram_parameter(
        "w_dr", [128, 2, K_PAIRS, 2, 128], fp8, isOutput=False)
    qt = nc.declare_dram_parameter("qt", [128, 2, BATCH], f32, isOutput=False)
    qtn = nc.declare_dram_parameter("qtn", [128, 2, BATCH], f32, isOutput=False)
    stair8 = nc.declare_dram_parameter(
        "stair8", [128, BATCH, 2, BATCH], fp8, isOutput=False)
    st_out = nc.declare_dram_parameter("st_out", [BATCH, SHARD_PAD], f32, isOutput=True)

    with tile.TileContext(nc) as tc:
        with (
            tc.tile_pool(name="const", bufs=1) as const_pool,
            tc.tile_pool(name="pt", bufs=3) as pt_pool,
            tc.tile_pool(name="tb", bufs=6) as tb_pool,
            tc.tile_pool(name="t8", bufs=8) as t8_pool,
            tc.tile_pool(name="sr", bufs=2) as sr_pool,
            tc.tile_pool(name="psumt", bufs=1, space="PSUM") as psumt_pool,
            tc.tile_pool(name="psums", bufs=2, space="PSUM") as psums_pool,
        ):
            # ---- resident constants ----
            w_sb = const_pool.tile([128, 2, K_PAIRS, 2, 128], fp8)
            nc.sync.dma_start(out=w_sb[:], in_=w_dr[:])
            qt_sb = const_pool.tile([128, 2, BATCH], f32)
            nc.sync.dma_start(out=qt_sb[:], in_=qt[:])
            qtn_sb = const_pool.tile([128, 2, BATCH], f32)
            nc.sync.dma_start(out=qtn_sb[:], in_=qtn[:])
            stair8_sb = const_pool.tile([128, BATCH, 2, BATCH], fp8)
            nc.sync.dma_start(out=stair8_sb[:], in_=stair8[:])
            # bf16 window stair: column BATCH-1 holds 2.0
            stairb = const_pool.tile([128, 2 * BATCH - 1], bf16)
            nc.gpsimd.memset(stairb[:], 0.0)
            nc.gpsimd.memset(stairb[:, BATCH - 1 : BATCH], 2.0)
            # negones: -1.0 columns for relu rows, 0 for abs rows
            negones = const_pool.tile([128, BATCH], bf16)
            nc.gpsimd.memset(negones[:], 0.0)
            nc.gpsimd.memset(negones[:, :N_RELU], -1.0)

            a_all = const_pool.tile([128, K_PAIRS, 2, SHARD_PAD], fp8)
            for g in range(N_GROUPS):
                nc.sync.dma_start(
                    out=a_all[:, :, :, GROUP * g : GROUP * (g + 1)],
                    in_=a_dr[:, :, :, GROUP * g : GROUP * (g + 1)],
                )

            gsizes = [GROUP] * (N_GROUPS - 1) + [SHARD - GROUP * (N_GROUPS - 1)]
            for g in range(N_GROUPS):
                g0 = g * GROUP
                gsz = gsizes[g]
                csz = [CHUNK] * (gsz // CHUNK) + ([gsz % CHUNK] if gsz % CHUNK else [])
                coff = [sum(csz[:i]) for i in range(len(csz))]
                nch = len(csz)

                # ---- fp8 DoubleRow projection: P.T halves [128d, gsz] ----
                pt_sb = pt_pool.tile([128, 2, GROUP], bf16, tag="pt")
                ptp = psumt_pool.tile([128, 2, GROUP], f32, tag="ptp")
                for h in range(2):
                    for c in range(nch):
                        for j in range(K_PAIRS):
                            nc.tensor.matmul(
                                ptp[:, h, coff[c] : coff[c] + csz[c]],
                                w_sb[:, h, j, :, :],
                                a_all[:, j, :, g0 + coff[c] : g0 + coff[c] + csz[c]],
                                start=(j == 0),
                                stop=(j == K_PAIRS - 1),
                                perf_mode=DR,
                            )
                # un-scale W_SCALE while converting to bf16 (both halves at once)
                nc.scalar.activation(
                    pt_sb[:, :, :gsz], ptp[:, :, :gsz], ACT.Copy,
                    bias=0.0, scale=1.0 / W_SCALE,
                )

                psum_s = psums_pool.tile([BATCH, GROUP], f32, tag="psum_s")
                started = [False] * nch
                n_dr_red = N_DVE8 + N_POOL + N_ACT
                n_bf_red = 2 * N_DVEB
                n_red_total = 2 * n_dr_red + (n_bf_red + 2) * nch  # per-chunk count bookkeeping below

                def red_dr(b, tile8):
                    nonlocal started
                    for c in range(nch):
                        nc.tensor.matmul(
                            psum_s[:, coff[c] : coff[c] + csz[c]],
                            stair8_sb[:, b],
                            tile8[:, :, coff[c] : coff[c] + csz[c]],
                            start=not started[c],
                            stop=False,
                            perf_mode=DR,
                            skip_group_check=True,
                        )
                        started[c] = True

                def red_bf(b, tileb, h):
                    nonlocal started
                    for c in range(nch):
                        nc.tensor.matmul(
                            psum_s[:, coff[c] : coff[c] + csz[c]],
                            stairb[:, BATCH - 1 - b : 2 * BATCH - 1 - b],
                            tileb[:, h, coff[c] : coff[c] + csz[c]],
                            start=not started[c],
                            stop=False,
                            skip_group_check=True,
                        )
                        started[c] = True

                # interleave rows across engines so every producer stays busy
                order = []
                mx = max(N_DVEB, N_DVE8, N_POOL, N_ACT)
                for i in range(mx):
                    for rows, kind in (
                        (ROWS_ACT, "act"),
                        (ROWS_DVEB, "dveb"),
                        (ROWS_POOL, "pool"),
                        (ROWS_DVE8, "dve8"),
                    ):
                        if i < len(rows):
                            order.append((rows[i], kind))

                for b, kind in order:
                    if kind == "dveb":
                        tl = tb_pool.tile([128, 2, GROUP], bf16, tag="tb")
                        for h in range(2):
                            nc.vector.tensor_scalar(
                                out=tl[:, h, :gsz],
                                in0=pt_sb[:, h, :gsz],
                                scalar1=qt_sb[:, h, b : b + 1],
                                scalar2=0.0,
                                op0=AL.subtract,
                                op1=AL.max,
                            )
                            red_bf(b, tl, h)
                    elif kind == "dve8":
                        tl = t8_pool.tile([128, 2, GROUP], fp8, tag="t8d")
                        for h in range(2):
                            nc.vector.tensor_scalar(
                                out=tl[:, h, :gsz],
                                in0=pt_sb[:, h, :gsz],
                                scalar1=qt_sb[:, h, b : b + 1],
                                scalar2=0.0,
                                op0=AL.subtract,
                                op1=AL.max,
                            )
                        red_dr(b, tl)
                    elif kind == "pool":
                        tl = t8_pool.tile([128, 2, GROUP], fp8, tag="t8p")
                        for h in range(2):
                            nc.gpsimd.tensor_scalar(
                                out=tl[:, h, :gsz],
                                in0=pt_sb[:, h, :gsz],
                                scalar1=qt_sb[:, h, b : b + 1],
                                scalar2=0.0,
                                op0=AL.subtract,
                                op1=AL.max,
                            )
                        red_dr(b, tl)
                    else:  # act
                        tl = t8_pool.tile([128, 2, GROUP], fp8, tag="t8a")
                        for h in range(2):
                            nc.scalar.activation(
                                tl[:, h, :gsz],
                                pt_sb[:, h, :gsz],
                                ACT.Abs,
                                bias=qtn_sb[:, h, b : b + 1],
                                scale=1.0,
                            )
                        red_dr(b, tl)

                # "- sum x" correction for relu rows (negones columns)
                for h in range(2):
                    for c in range(nch):
                        nc.tensor.matmul(
                            psum_s[:, coff[c] : coff[c] + csz[c]],
                            negones[:],
                            pt_sb[:, h, coff[c] : coff[c] + csz[c]],
                            start=False,
                            stop=(h == 1),
                            skip_group_check=True,
                        )

                sr = sr_pool.tile([BATCH, GROUP], f32, tag="sr")
                nc.scalar.copy(sr[:, :gsz], psum_s[:, :gsz])
                nc.sync.dma_start(out=st_out[:, g0 : g0 + gsz], in_=sr[:, :gsz])

    nc.compile()
    return nc


def _get_program():
    if "nc" not in _CACHE:
        _CACHE["nc"] = _build_program()
    return _CACHE["nc"]


def _host_query_sum(ent_pkl, other_emb, proj_W, batch_input_ids, mp):
    """Exact replica of the reference's query path, on host (64 rows only)."""
    ids = np.concatenate([batch_input_ids[:, :mp], batch_input_ids[:, mp + 1 : 3]], axis=1)
    ids = ids.astype(np.int64)  # [B, 2]
    q = np.empty((BATCH, 2, EMBED_DIM), dtype=np.float32)
    for b in range(BATCH):
        for j in range(2):
            idx = int(ids[b, j])
            if idx == 0:
                row = other_emb[0]
            elif idx <= NUM_ENT:
                row = ent_pkl[idx - 1].astype(np.float32) @ proj_W.T.astype(np.float32)
            else:
                row = other_emb[idx - NUM_ENT]
            q[b, j] = row
    norm = np.sqrt((q * q).sum(-1, keepdims=True))
    q = q / np.maximum(norm, EPS)
    return q.sum(axis=1)  # [B, 256] float32


def kernel(ent_pkl, other_emb, proj_W, batch_input_ids, batch_mask_position, _timing=None):
    from concourse.bass_utils import run_bass_kernel_spmd

    ent_pkl = np.asarray(ent_pkl, dtype=np.float32)
    other_emb = np.asarray(other_emb, dtype=np.float32)
    proj_W = np.asarray(proj_W, dtype=np.float32)
    batch_input_ids = np.asarray(batch_input_ids)
    mp = int(np.asarray(batch_mask_position))

    q_sum = _host_query_sum(ent_pkl, other_emb, proj_W, batch_input_ids, mp)

    # score column 0: entity row = other_emb[0]
    col0 = -np.abs(q_sum - other_emb[0][None, :]).sum(-1)  # [B]

    # ---- device input prep ----
    # w_dr[kp, h, j, i, m] = W_SCALE * proj_W.T[128*(2j+i)+kp, 128h+m]
    w_t = np.ascontiguousarray(proj_W.T) * W_SCALE  # [768, 256]
    w_resh = w_t.reshape(K_PAIRS, 2, 128, 2, 128)   # [j, i, kp, h, m]
    w_dr_np = np.ascontiguousarray(
        w_resh.transpose(2, 3, 0, 1, 4)).astype(FP8)  # [128, 2, 3, 2, 128]

    # qt[kp, h, b] = q_sum[b, 128h+kp]
    qth = np.transpose(q_sum.T.reshape(2, 128, BATCH), (1, 0, 2))  # [128, 2, 32]
    qt_np = np.ascontiguousarray(qth.astype(np.float32))
    qtn_np = np.ascontiguousarray((-qth).astype(np.float32))

    # per-row DoubleRow stair: stair8[:, b, :, b] = 2.0 (relu rows) / 1.0 (abs)
    stair8_np = np.zeros((128, BATCH, 2, BATCH), dtype=FP8)
    for b in range(BATCH):
        stair8_np[:, b, :, b] = FP8(2.0) if b < N_RELU else FP8(1.0)

    # a_dr[kp, j, i, e] = ent_shard.T[128*(2j+i)+kp, e]
    a_t_full = ent_pkl.T.astype(FP8)  # [768, 40000]
    in_maps = []
    for c in range(N_CORES):
        shard_t = a_t_full[:, c * SHARD : (c + 1) * SHARD]  # [768, 5000] fp8
        a_np = np.zeros((128, K_PAIRS, 2, SHARD_PAD), dtype=FP8)
        a_np[:, :, :, :SHARD] = shard_t.reshape(
            K_PAIRS, 2, 128, SHARD).transpose(2, 0, 1, 3)
        in_maps.append({
            "a_dr": a_np,
            "w_dr": w_dr_np,
            "qt": qt_np,
            "qtn": qtn_np,
            "stair8": stair8_np,
        })

    nc = _get_program()
    kwargs = dict(_timing) if _timing else {}
    res = run_bass_kernel_spmd(nc, in_maps, list(range(N_CORES)), **kwargs)
    if _timing is not None:
        _CACHE["last_results"] = res

    qsum = q_sum.sum(-1).astype(np.float32)  # [B]
    s_ent = np.empty((BATCH, NUM_ENT), dtype=np.float32)
    for c in range(N_CORES):
        sl = slice(c * SHARD, (c + 1) * SHARD)
        s_ent[:, sl] = res.results[c]["st_out"][:, :SHARD]
    # relu-identity rows still need the +sum(q) term
    s_ent[:N_RELU] += qsum[:N_RELU, None]
    out = np.empty((BATCH, NUM_ENT + 1), dtype=np.float32)
    out[:, 0] = col0
    out[:, 1:] = -s_ent
    return out
